# revision 71
# baseline (speedup 1.0000x reference)
"""Trainium2 Bass kernel for nn_LogicAutoEncoder.

Math: board_state (B,9,3) one-hot -> logits (B,9,3).
  sim[b,r,p,i] = exp(-d2[r,p,i,c(b,i)]) where c is the cell state, so
  sat[b,r,p] = max_i sim is replaced by the beta-power-mean
    sat^beta = sum_i exp(-beta*d2[r,p,i,c_i])  (exact one-hot matmul),
  which for beta=32 matches max_i to ~1e-2 relative (fro) accuracy on hw.
  act[b,r] = sat0*sat1 = exp((ln A0 + ln A1)/beta) * C_r with the
  per-premise table shift folded into scaled heads rows.

Device pipeline (pure data parallel over 8 cores, 65536 rows each), per
8192-row supertile, with the one-hot input pre-transposed on host into
4-row-group slabs (108 = 4x27 partitions):
  1. one DMA in (108, 2048) bf16
  2. 16 block-diag matmuls (108,128)^T @ W4 (108,64) -> A (128,1024) PSUM
  3. ScalarE Ln -> L (f32), DVE add premise pairs -> Ls, ScalarE
     Exp(scale=1/beta) -> act (f16, with a ones column for the bias row)
  4. PE transposes act in 3-chunk groups -> PSUM f16, DVE copy
  5. 6 block-diag heads matmuls (scaled heads + bias row) -> PSUM f32
  6. f32->f16 copies split across Scalar/DVE/GpSimd engines, one DMA out.
"""

import functools
import os
import sys

import numpy as np

sys.path.insert(0, "/opt/trn_rl_repo")

import ml_dtypes

B = 524288
N_CORES = 8
BC = B // N_CORES            # 65536 rows per core
ST_ROWS = 8192               # rows per supertile
N_ST = BC // ST_ROWS         # 8 supertiles
N_T = 16                     # 512-row chunks per supertile
BETA = 32.0
CENTER = 41.0
EPS_LN = -43.0        # table flush / epsilon floor, inside the HW Ln range

COLS = N_ST * N_T * 128      # 16384 columns of the transposed input


DEFAULT_CFG = {
    "pA_bufs": 1,       # (128,1024) f32 = 2 banks each
    "pa_bufs": 1,       # (108,768) bf16 = 1 bank each
    "po_bufs": 2,       # (128,324) f32 = 1 bank each (single) / 2 (pair)
    "po_pair": True,    # heads outputs grouped in pairs
    "cp": "vsv",        # copy engines per group: s=scalar v=vector (PSUM read)
    "aT_eng": "v",      # engine for the aT PSUM->SBUF copy
    "add_eng": "v",     # engine for the SBUF pair-add (g=gpsimd frees DVE)
    "off_s3": 1,        # slot offset of the pair-add
    "off_ta": 3,        # slot offset of tail_a
    "off_tb": 4,        # slot offset of tail_b
    "in_dt": "fp8",    # input one-hot dtype: bf16 or fp8 (float8e4)
    "l_bufs": 2,
    "ls_bufs": 3,
    "act_bufs": 2,
    "aTs_bufs": 2,
    "out_bufs": 4,
}


def _build_program(cfg=None):
    import concourse.bacc as bacc
    import concourse.mybir as mybir
    import concourse.tile as tile

    cfg = {**DEFAULT_CFG, **(cfg or {})}

    f32 = mybir.dt.float32
    f16 = mybir.dt.float16
    bf16 = mybir.dt.bfloat16
    AF = mybir.ActivationFunctionType

    nc = bacc.Bacc(
        "TRN2", target_bir_lowering=False, debug=False, num_devices=N_CORES
    )
    in_dt = bf16 if cfg["in_dt"] == "bf16" else mybir.dt.float8e4
    xt_d = nc.dram_tensor("xt", [112, COLS], in_dt, kind="ExternalInput")
    w4_d = nc.dram_tensor("w4", [112, 64], bf16, kind="ExternalInput")
    hb_d = nc.dram_tensor("hb", [108, 324], bf16, kind="ExternalInput")
    idm_d = nc.dram_tensor("idm", [128, 128], f16, kind="ExternalInput")
    out_d = nc.dram_tensor("out", [BC, 27], f16, kind="ExternalOutput")

    out_v = out_d.rearrange("(s m k) f -> s m (k f)", s=N_ST, m=128, k=64)

    # Pre-place the activation-table load for the set containing ln, exp
    # and copy, so the compiler's fixpoint inserts no per-iteration reloads.
    act_set_id = 6  # natural_log_exp_and_others in act_info.json
    try:
        from concourse.hw_specs import get_activation_tables

        tables = list(get_activation_tables(nc.m.arch).items())
        for idx, (name, funcs) in enumerate(tables):
            fs = {str(f).split(".")[-1] for f in funcs}
            if {"Ln", "Exp", "Copy"} <= fs:
                act_set_id = idx
                break
    except Exception:
        pass

    with tile.TileContext(nc) as tc:
        nc.scalar.add_instruction(
            mybir.InstLoadActFuncSet(
                name=f"I-{nc.next_id()}", ins=[], outs=[], act_func_set_id=act_set_id
            )
        )
        with (
            tc.tile_pool(name="singles", bufs=1) as singles,
            tc.tile_pool(name="bs_in", bufs=N_ST) as bs_pool,
            tc.tile_pool(name="lt", bufs=cfg["l_bufs"]) as l_pool,
            tc.tile_pool(name="lst", bufs=cfg["ls_bufs"]) as ls_pool,
            tc.tile_pool(name="act", bufs=cfg["act_bufs"]) as act_pool,
            tc.tile_pool(name="aT", bufs=cfg["aTs_bufs"]) as aT_pool,
            tc.tile_pool(name="out_sb", bufs=cfg["out_bufs"]) as out_pool,
            tc.tile_pool(name="p_A", bufs=cfg["pA_bufs"], space="PSUM") as p_A,
            tc.tile_pool(name="p_aT", bufs=cfg["pa_bufs"], space="PSUM") as p_aT,
            tc.tile_pool(name="p_o", bufs=cfg["po_bufs"], space="PSUM") as p_o,
        ):
            state = {}

            def dma_in(s):
                bs = bs_pool.tile([112, N_T * 128], in_dt)
                nc.sync.dma_start(
                    out=bs[:], in_=xt_d[:, s * 2048 : (s + 1) * 2048]
                )
                state[("bs", s)] = bs

            def mm1(s):
                bs = state.pop(("bs", s))
                state[("bs_done", s)] = bs
                A = p_A.tile([128, N_T * 64], f32)
                for t in range(N_T):
                    nc.tensor.matmul(
                        A[:, t * 64 : (t + 1) * 64],
                        bs[:, t * 128 : (t + 1) * 128],
                        w4_sb[:],
                        start=True,
                        stop=True,
                    )
                # ln(A) straight out of PSUM (only one PSUM operand is legal
                # per vector op, so the pair-product happens after the logs,
                # in SBUF, as an add).
                P = l_pool.tile([128, N_T * 64], f32)
                nc.scalar.activation(P[:], A[:], AF.Ln)
                state[("P", s)] = P

            def stage3(s):
                P = state.pop(("P", s))
                L2 = ls_pool.tile([128, N_T * 32], f32)
                Lv = P[:].rearrange(
                    "a (t g p r) -> a t g p r", t=N_T, g=4, p=2
                )
                L2v = L2[:].rearrange("a (t g r) -> a t g r", t=N_T, g=4)
                add_eng = nc.gpsimd if cfg["add_eng"] == "g" else nc.vector
                add_eng.tensor_add(L2v, Lv[:, :, :, 0, :], Lv[:, :, :, 1, :])
                state[("L2", s)] = L2

            def tail_a(s):
                L2 = state.pop(("L2", s))
                Lsv = L2[:].rearrange("a (t g r) -> a t g r", t=N_T, g=4)

                act = act_pool.tile([128, N_T, 4, 9], f16)
                if s < cfg["act_bufs"]:
                    nc.gpsimd.memset(act[:, :, :, 8:9], 1.0)
                nc.scalar.activation(
                    act[:, :, :, 0:8], Lsv, AF.Exp, scale=1.0 / BETA
                )
                act2 = act[:].rearrange("a t g n -> a (t g n)")

                pa = p_aT.tile([108, 768], f16)
                for i in range(5):
                    nc.tensor.transpose(
                        pa[:, i * 128 : (i + 1) * 128],
                        act2[:, i * 108 : (i + 1) * 108],
                        idm_sb[:],
                    )
                nc.tensor.transpose(
                    pa[0:36, 640:768], act2[:, 540:576], idm_sb[:]
                )
                aT = aT_pool.tile([108, 768], bf16)
                eng_map = {
                    "s": nc.scalar.copy,
                    "v": nc.vector.tensor_copy,
                    "g": nc.gpsimd.tensor_copy,
                }
                eng_map[cfg["aT_eng"]](aT[:], pa[:])

                out_sb = out_pool.tile([128, N_T * 108], f16)
                cp_engine = [eng_map[c] for c in cfg["cp"]]

                def head_mm(po, poff, i):
                    # PSUM matmul outputs must not straddle a 2KB bank
                    # boundary: start each at an f32 offset of 0 or 512.
                    if i < 5:
                        nc.tensor.matmul(
                            po[:, poff : poff + 324],
                            aT[:, i * 128 : (i + 1) * 128],
                            hb_sb[:],
                            start=True,
                            stop=True,
                        )
                    else:
                        nc.tensor.matmul(
                            po[:, poff : poff + 108],
                            aT[0:36, 640:768],
                            hb_sb[0:36, 0:108],
                            start=True,
                            stop=True,
                        )

                if cfg["po_pair"]:
                    for h in range(3):
                        po = p_o.tile([128, 1024], f32)
                        head_mm(po, 0, 2 * h)
                        head_mm(po, 512, 2 * h + 1)
                        pov = po[:].rearrange("a (c n) -> a c n", c=2)
                        if h < 2:
                            dst = out_sb[:, h * 648 : (h + 1) * 648]
                            cp_engine[h](
                                dst.rearrange("a (c n) -> a c n", c=2),
                                pov[:, :, 0:324],
                            )
                        else:
                            cp_engine[h](
                                out_sb[:, 1296:1620], po[:, 0:324]
                            )
                            cp_engine[3 % len(cp_engine)](
                                out_sb[:, 1620:1728], po[:, 512:620]
                            )
                else:
                    for i in range(6):
                        po = p_o.tile([128, 512], f32)
                        head_mm(po, 0, i)
                        n = 324 if i < 5 else 108
                        cp_engine[i](
                            out_sb[:, i * 324 : i * 324 + n], po[:, 0:n]
                        )
                state[("out", s)] = out_sb

            def tail_b(s):
                out_sb = state.pop(("out", s))
                nc.sync.dma_start(out=out_v[s], in_=out_sb[:])

            for s in range(4):
                dma_in(s)
            w4_sb = singles.tile([112, 64], bf16)
            nc.gpsimd.dma_start(out=w4_sb[:], in_=w4_d[:])
            hb_sb = singles.tile([108, 324], bf16)
            nc.gpsimd.dma_start(out=hb_sb[:], in_=hb_d[:])
            idm_sb = singles.tile([128, 128], f16)
            nc.gpsimd.dma_start(out=idm_sb[:], in_=idm_d[:])

            n_shallow = cfg.get("n_shallow", 0)

            def ta_slot(u):
                return u + (2 if u < n_shallow else cfg["off_ta"])

            for s in range(N_ST + cfg["off_tb"]):
                if s < N_ST:
                    mm1(s)
                for u in range(N_ST):
                    if ta_slot(u) == s:
                        tail_a(u)
                if 0 <= s - cfg["off_s3"] < N_ST:
                    stage3(s - cfg["off_s3"])
                for u in range(N_ST):
                    if ta_slot(u) + 1 == s:
                        tail_b(u)
                if 4 <= s + 4 < N_ST:
                    dma_in(s + 4)

    nc.compile()
    return nc


@functools.cache
def _get_program():
    return _build_program()


def _host_tables(premises, heads, bias):
    """Build the shifted beta-power tables and scaled heads on host (tiny)."""
    pos = (np.arange(9, dtype=np.float64) - 4.0) / 4.0
    pl = np.array([0.0, 1.0, -1.0], dtype=np.float64)
    prem = premises.astype(np.float64)
    d2 = (pl[None, None, None, :] - prem[:, :, None, None, 0]) ** 2 + (
        pos[None, None, :, None] - prem[:, :, None, None, 1]
    ) ** 2  # (r, p, i, c)
    shift = d2.min(axis=(2, 3)) + CENTER / BETA  # (r, p)
    W = np.exp(-BETA * (d2 - shift[:, :, None, None]))  # (r,p,i,c)
    # Entries below the epsilon floor would push ln(A) outside the HW Ln
    # table's valid range (|ln| <~ 44); flush them and add an epsilon row
    # instead so A in [e^EPS_LN, 9*e^CENTER] always.
    W[W < np.exp(EPS_LN)] = 0.0

    w4 = np.zeros((112, 64), dtype=np.float32)
    wt = W.transpose(2, 3, 1, 0).reshape(27, 16)  # [(i,c), (p,r)]
    for g in range(4):
        w4[g * 28 : g * 28 + 27, g * 16 : (g + 1) * 16] = wt
        w4[g * 28 + 27, g * 16 : (g + 1) * 16] = np.exp(EPS_LN)

    C = np.exp(-shift.sum(axis=-1))  # (r,)
    heads9 = np.zeros((9, 27), dtype=np.float64)
    heads9[0:8] = heads.astype(np.float64) * C[:, None]
    heads9[8] = bias.astype(np.float64)
    hb = np.zeros((108, 324), dtype=np.float32)
    for gg in range(12):
        hb[gg * 9 : (gg + 1) * 9, gg * 27 : (gg + 1) * 27] = heads9

    bf = ml_dtypes.bfloat16
    return w4.astype(bf), hb.astype(bf), np.eye(128, dtype=np.float16)


def kernel(board_state, premises, heads, bias):
    from concourse.bass_utils import run_bass_kernel_spmd

    nc = _get_program()
    w4, hb, idm = _host_tables(
        np.asarray(premises), np.asarray(heads), np.asarray(bias)
    )

    in_np = (
        ml_dtypes.bfloat16
        if DEFAULT_CFG["in_dt"] == "bf16"
        else ml_dtypes.float8_e4m3
    )
    x = np.ascontiguousarray(board_state, dtype=np.float32).reshape(B, 27)
    # row = core*BC + s*8192 + m*64 + t*4 + g ; xt[core][(g,f), (s,t,m)]
    # plus a ones row per 28-row group (the epsilon floor for ln).
    xv = x.reshape(N_CORES, N_ST, 128, N_T, 4, 27)
    xg = np.ones((N_CORES, 4, 28, N_ST, N_T, 128), dtype=np.float32)
    xg[:, :, 0:27] = xv.transpose(0, 4, 5, 1, 3, 2)
    xt = np.ascontiguousarray(xg).reshape(N_CORES, 112, COLS).astype(in_np)

    in_maps = []
    for k in range(N_CORES):
        in_maps.append({"xt": xt[k], "w4": w4, "hb": hb, "idm": idm})
    res = run_bass_kernel_spmd(
        nc,
        in_maps,
        core_ids=list(range(N_CORES)),
        trace=bool(int(os.environ.get("KERNEL_TRACE", "0"))),
    )
    out = np.concatenate([r["out"] for r in res.results], axis=0)
    kernel.last_results = res
    return out.astype(np.float32).reshape(B, 9, 3)
